# revision 1
# baseline (speedup 1.0000x reference)
"""EventDenoisingMamba Trainium2 kernel.

Data-parallel over batch: 8 batch elements -> 8 NeuronCores, one full
sequence (S=8192) per core. Layout on chip: channels on partitions, time on
the free dimension.

Per layer:
  - in_proj + causal depthwise conv folded into 4 PSUM-accumulated matmuls
    (host precomputes K[d,(k,c)] = W_u[d,c]*conv_w[c,k]; x is zero-padded by
    3 columns at the sequence start).
  - x_proj / dt_proj on PE, softplus/silu/exp on ACT.
  - selective scan h[t] = exp(delta*A)*h[t-1] + delta*u*B via
    tensor_tensor_scan per (d-block, state) stream, chained across time
    chunks through a carry column.
  - B/C row broadcasts via K=1 matmuls into PSUM, consumed directly by DVE.
"""

import numpy as np

import concourse.bass as bass
import concourse.tile as tile
from concourse import bacc, mybir

F32 = mybir.dt.float32
BF16 = mybir.dt.bfloat16
AF = mybir.ActivationFunctionType
OP = mybir.AluOpType

S = 8192
DM = 128      # d_model
DI = 256      # d_inner
NST = 16      # d_state
DC = 4        # d_conv
RK = 8        # dt_rank
NL = 4        # layers
NCORES = 8


class Ctx:
    pass


def _load_weights(c, nc, drams):
    wp = c.wp
    (kuc, wz, xpw, dtw, ow, emb, headw, ones, dtb, cb, aneg, dpar, embb,
     headb, featT) = drams
    c.w_kuc, c.w_wz, c.w_xpw, c.w_dtw, c.w_ow = [], [], [], [], []
    c.w_dtb, c.w_cb, c.w_a, c.w_d = [], [], [], []
    for l in range(NL):
        for lst, dram, shape, dt in [
            (c.w_kuc, kuc, [128, DC * DI], BF16),
            (c.w_wz, wz, [128, DI], BF16),
            (c.w_xpw, xpw, [128, 80], BF16),
            (c.w_dtw, dtw, [RK, DI], BF16),
            (c.w_ow, ow, [128, 256], BF16),
            (c.w_dtb, dtb, [128, 2], F32),
            (c.w_cb, cb, [128, 2], F32),
            (c.w_a, aneg, [128, 2 * NST], F32),
            (c.w_d, dpar, [128, 2], F32),
        ]:
            t = wp.tile(shape, dt, tag=f"w{len(lst)}_{id(lst) % 997}", name=f"w{len(lst)}_{id(lst) % 997}")
            nc.sync.dma_start(t, dram[l])
            lst.append(t)
    c.w_emb = wp.tile([11, DM], BF16, tag="emb", name="emb")
    nc.sync.dma_start(c.w_emb, emb[:])
    c.w_headw = wp.tile([DM, 1], BF16, tag="headw", name="headw")
    nc.sync.dma_start(c.w_headw, headw[:])
    c.w_embb = wp.tile([128, 1], F32, tag="embb", name="embb")
    nc.sync.dma_start(c.w_embb, embb[:])
    c.w_headb = wp.tile([1, 1], F32, tag="headb", name="headb")
    nc.sync.dma_start(c.w_headb, headb[:])


def _embed(c, nc):
    for blk in range(c.s // c.bw):
        ps = c.pp.tile([128, c.bw], F32, tag="mm", name="mm")
        for h in range(c.bw // 512):
            col = blk * c.bw + h * 512
            nc.tensor.matmul(
                ps[:, h * 512:(h + 1) * 512],
                lhsT=c.w_emb, rhs=c.w_feat[:, col:col + 512],
                start=True, stop=True)
        nc.scalar.activation(
            c.xa[:, 3 + blk * c.bw: 3 + (blk + 1) * c.bw],
            ps, AF.Identity, bias=c.w_embb[:, 0:1])


def _uz(c, nc, l, xin, t0, db, blk):
    bw = c.bw
    ps = c.pp.tile([128, bw], F32, tag="mm", name="mm")
    for h in range(bw // 512):
        col = t0 + blk * bw + h * 512
        for k in range(DC):
            nc.tensor.matmul(
                ps[:, h * 512:(h + 1) * 512],
                lhsT=c.w_kuc[l][:, k * DI + db * 128:k * DI + db * 128 + 128],
                rhs=xin[:, col + k:col + k + 512],
                start=(k == 0), stop=(k == DC - 1))
    off = db * c.tc_len + blk * bw
    nc.scalar.activation(
        c.u_sb[:, off:off + bw], ps, AF.Silu,
        bias=c.w_cb[l][:, db:db + 1])
    ps = c.pp.tile([128, bw], F32, tag="mm", name="mm")
    for h in range(bw // 512):
        col = t0 + blk * bw + h * 512
        nc.tensor.matmul(
            ps[:, h * 512:(h + 1) * 512],
            lhsT=c.w_wz[l][:, db * 128:db * 128 + 128],
            rhs=xin[:, 3 + col:3 + col + 512],
            start=True, stop=True)
    nc.scalar.activation(
        c.zs_sb[:, off:off + bw], ps, AF.Silu)


def _xdbl(c, nc, l, blk):
    bw = c.bw
    ps = c.pp.tile([128, bw], F32, tag="mm", name="mm")
    for h in range(bw // 512):
        col = blk * bw + h * 512
        for ct in range(2):
            nc.tensor.matmul(
                ps[0:40, h * 512:(h + 1) * 512],
                lhsT=c.w_xpw[l][:, ct * 40:ct * 40 + 40],
                rhs=c.u_sb[:, ct * c.tc_len + col:ct * c.tc_len + col + 512],
                start=(ct == 0), stop=(ct == 1))
    nc.scalar.activation(
        c.xd_sb[:, blk * bw:(blk + 1) * bw], ps[0:40, :], AF.Copy)


def _delta(c, nc, l, db, blk):
    bw = c.bw
    ps = c.pp.tile([128, bw], F32, tag="mm", name="mm")
    for h in range(bw // 512):
        col = blk * bw + h * 512
        nc.tensor.matmul(
            ps[:, h * 512:(h + 1) * 512],
            lhsT=c.w_dtw[l][:, db * 128:db * 128 + 128],
            rhs=c.xd_sb[0:RK, col:col + 512],
            start=True, stop=True)
    # softplus(x) = relu(x) + ln(1 + exp(-|x|)) (stable; Softplus has no
    # ACT table entry on this compiler).
    ab = c.tmpp.tile([128, bw], F32, tag="tm", name="spa")
    nc.scalar.activation(ab, ps, AF.Abs, bias=c.w_dtb[l][:, db:db + 1])
    rl = c.tmpp.tile([128, bw], BF16, tag="tmh", name="spr")
    nc.scalar.activation(rl, ps, AF.Relu, bias=c.w_dtb[l][:, db:db + 1])
    ab2 = c.tmpp.tile([128, bw], BF16, tag="tmh", name="spe")
    nc.scalar.activation(ab2, ab, AF.Exp, scale=-1.0)
    nc.scalar.activation(ab2, ab2, AF.Ln, bias=1.0)
    off = db * c.tc_len + blk * bw
    nc.vector.tensor_tensor(c.de_sb[:, off:off + bw], rl, ab2, OP.add)


def _ssm(c, nc, l, bcd_r):
    """Scan + y for one chunk. B/C rows bounce through DRAM and come back
    as partition-broadcast DMA reads (bf16), so the elementwise multiplies
    run in DVE 2x mode with zero engine cost for the broadcast."""
    tc_len = c.tc_len
    t2 = 2 * tc_len
    nc.gpsimd.dma_start(bcd_r, c.xd_sb[RK:RK + 2 * NST, :])
    tm_hold = None
    c.y_gp = [None, None]
    for n in range(NST):
        bb = c.bcp.tile([128, tc_len], BF16, tag="bb", name="bb")
        cb2 = c.bcp.tile([128, tc_len], BF16, tag="cb2", name="cb2")
        for j, (row, dst) in enumerate(((n, bb), (NST + n, cb2))):
            srow = bcd_r[row:row + 1, :]
            bcast = bass.AP(tensor=srow.tensor, offset=srow.offset,
                            ap=[[0, 128]] + [list(x) for x in srow.ap[1:]])
            qeng = (nc.sync, nc.scalar)[j]
            qeng.dma_start(dst, bcast)
        # dbu for both d-blocks in one op (bb read twice via step-0 AP)
        dbu_t = c.dbup.tile([128, t2], BF16, tag="dbu", name="dbu")
        bb2 = bass.AP(tensor=bb.tensor, offset=bb.offset,
                      ap=[list(bb.ap[0]), [0, 2]] + [list(x) for x in bb.ap[1:]])
        deng = nc.gpsimd if n % 4 == 3 else nc.vector
        deng.tensor_tensor(
            dbu_t.rearrange("p (b t) -> p b t", b=2), c.du_sb.rearrange(
                "p (b t) -> p b t", b=2), bb2, OP.mult)
        h_t = c.hp.tile([128, t2], BF16, tag="h", name="h")
        for db in range(2):
            da_t = c.dap.tile([128, tc_len], F32, tag="da", name="da")
            nc.scalar.activation(
                da_t, c.de_sb[:, db * tc_len:(db + 1) * tc_len], AF.Exp,
                scale=c.w_a[l][:, db * NST + n:db * NST + n + 1])
            nc.vector.tensor_tensor_scan(
                h_t[:, db * tc_len:(db + 1) * tc_len], da_t,
                dbu_t[:, db * tc_len:(db + 1) * tc_len],
                initial=c.hcm[:, db * NST + n:db * NST + n + 1],
                op0=OP.mult, op1=OP.add)
        hsrc = bass.AP(tensor=h_t.tensor, offset=h_t.offset + tc_len - 1,
                       ap=[list(h_t.ap[0]), [tc_len, 2]])
        nc.vector.tensor_copy(
            c.hcm.rearrange("p (b n) -> p b n", b=2)[:, :, n:n + 1], hsrc)
        cc2 = bass.AP(tensor=cb2.tensor, offset=cb2.offset,
                      ap=[list(cb2.ap[0]), [0, 2]] + [list(x) for x in cb2.ap[1:]])
        if n == 0:
            nc.vector.tensor_tensor(
                c.y_sb.rearrange("p (b t) -> p b t", b=2),
                h_t.rearrange("p (b t) -> p b t", b=2), cc2, OP.mult)
            continue
        tm = c.tmpp.tile([128, t2], BF16, tag="tmb", name="tm", bufs=6)
        nc.vector.tensor_tensor(
            tm.rearrange("p (b t) -> p b t", b=2),
            h_t.rearrange("p (b t) -> p b t", b=2), cc2, OP.mult)
        if n % 2 == 1:
            tm_hold = tm
            continue
        pair = c.tmpp.tile([128, t2], BF16, tag="tmb", name="pair", bufs=6)
        nc.vector.tensor_tensor(pair, tm_hold, tm, OP.add)
        tm_hold = None
        k = ((n // 2) - 1) % 2
        if c.y_gp[k] is None:
            c.y_gp[k] = pair
        else:
            yg2 = c.tmpp.tile([128, t2], BF16, tag=f"ygp{k}", name="yg2",
                              bufs=2)
            nc.gpsimd.tensor_tensor(yg2, c.y_gp[k], pair, OP.add)
            c.y_gp[k] = yg2
    if tm_hold is not None:
        nc.vector.tensor_tensor(c.y_sb, c.y_sb, tm_hold, OP.add)
    nc.vector.tensor_tensor(c.y_gp[0], c.y_gp[0], c.y_gp[1], OP.add)
    nc.vector.tensor_tensor(c.y_sb, c.y_sb, c.y_gp[0], OP.add)


def _outproj(c, nc, l, xout, t0, yg, blk):
    bw = c.bw
    ps = c.pp.tile([128, bw], F32, tag="mm", name="mm")
    for h in range(bw // 512):
        col = blk * bw + h * 512
        for ct in range(2):
            nc.tensor.matmul(
                ps[:, h * 512:(h + 1) * 512],
                lhsT=c.w_ow[l][:, ct * 128:ct * 128 + 128],
                rhs=yg[:, ct * c.tc_len + col:ct * c.tc_len + col + 512],
                start=(ct == 0), stop=(ct == 1))
    nc.scalar.activation(
        xout[:, 3 + t0 + blk * bw:3 + t0 + (blk + 1) * bw], ps, AF.Copy)


def _layer(c, nc, l, bcd):
    xin = c.xa if l % 2 == 0 else c.xb
    xout = c.xb if l % 2 == 0 else c.xa
    hc = c.hcp.tile([128, 2 * NST], F32, tag="hc", name="hc")
    nc.vector.memset(hc, 0.0)
    c.hcm = hc

    t2 = 2 * c.tc_len
    for ci in range(c.nch):
        t0 = ci * c.tc_len
        c.u_sb = c.ubufp.tile([128, t2], BF16, tag="u", name="u")
        c.zs_sb = c.zbufp.tile([128, t2], BF16, tag="z", name="z")
        c.de_sb = c.dbufp.tile([128, t2], BF16, tag="de", name="de")
        c.du_sb = c.dubufp.tile([128, t2], BF16, tag="du", name="du")
        c.y_sb = c.ybufp.tile([128, t2], BF16, tag="y", name="y")
        c.xd_sb = c.xdblp.tile([40, c.tc_len], BF16, tag="xd", name="xd")

        for db in range(2):
            for blk in range(c.nblk):
                _uz(c, nc, l, xin, t0, db, blk)
        for blk in range(c.nblk):
            _xdbl(c, nc, l, blk)
        for db in range(2):
            for blk in range(c.nblk):
                _delta(c, nc, l, db, blk)
        nc.vector.tensor_tensor(c.du_sb, c.de_sb, c.u_sb, OP.mult)

        _ssm(c, nc, l, bcd[(l * c.nch + ci) % 4])

        yg = c.ygatep.tile([128, t2], BF16, tag="yg", name="yg")
        for db in range(2):
            sl = slice(db * c.tc_len, (db + 1) * c.tc_len)
            nc.vector.scalar_tensor_tensor(
                c.y_sb[:, sl], c.u_sb[:, sl], c.w_d[l][:, db:db + 1],
                c.y_sb[:, sl], OP.mult, OP.add)
        nc.vector.tensor_tensor(yg, c.y_sb, c.zs_sb, OP.mult)
        for blk in range(c.nblk):
            _outproj(c, nc, l, xout, t0, yg, blk)


def _head(c, nc, out):
    xfin = c.xa if NL % 2 == 0 else c.xb
    for blk in range(c.s // c.bw):
        ps = c.pp.tile([128, c.bw], F32, tag="mm", name="mm")
        for h in range(c.bw // 512):
            col = blk * c.bw + h * 512
            nc.tensor.matmul(
                ps[0:1, h * 512:(h + 1) * 512],
                lhsT=c.w_headw, rhs=xfin[:, 3 + col:3 + col + 512],
                start=True, stop=True)
        ot = c.tmpp.tile([128, c.bw], F32, tag="tm", name="ot")
        nc.scalar.activation(ot[0:1, :], ps[0:1, :], AF.Sigmoid,
                             bias=c.w_headb[0:1, 0:1])
        nc.sync.dma_start(out[0:1, blk * c.bw:(blk + 1) * c.bw], ot[0:1, :])


def build(s=S, tc_len=1024, nloops=1):
    nc = bacc.Bacc("TRN2", target_bir_lowering=False, debug=False,
                   num_devices=NCORES)
    drams = (
        nc.declare_dram_parameter("kuc", [NL, 128, DC * DI], BF16, False),
        nc.declare_dram_parameter("wz", [NL, 128, DI], BF16, False),
        nc.declare_dram_parameter("xpw", [NL, 128, 80], BF16, False),
        nc.declare_dram_parameter("dtw", [NL, RK, DI], BF16, False),
        nc.declare_dram_parameter("ow", [NL, 128, 256], BF16, False),
        nc.declare_dram_parameter("emb", [11, DM], BF16, False),
        nc.declare_dram_parameter("headw", [DM, 1], BF16, False),
        nc.declare_dram_parameter("ones", [40, 32 * 128], BF16, False),
        nc.declare_dram_parameter("dtb", [NL, 128, 2], F32, False),
        nc.declare_dram_parameter("cb", [NL, 128, 2], F32, False),
        nc.declare_dram_parameter("aneg", [NL, 128, 2 * NST], F32, False),
        nc.declare_dram_parameter("dpar", [NL, 128, 2], F32, False),
        nc.declare_dram_parameter("embb", [128, 1], F32, False),
        nc.declare_dram_parameter("headb", [1, 1], F32, False),
    )
    featT = nc.declare_dram_parameter("featT", [11, s], BF16, False)
    out = nc.declare_dram_parameter("out", [1, s], F32, True)
    bcd = nc.dram_tensor("bcd", [4, 2 * NST, tc_len], BF16)
    drams = tuple(list(drams[:14]) + [featT])
    # reorder: _load_weights expects (kuc..headb, featT)

    c = Ctx()
    c.s = s
    c.tc_len = tc_len
    c.nch = s // tc_len
    c.bw = min(tc_len, 1024)
    c.nblk = tc_len // c.bw

    with tile.TileContext(nc) as tcx:
        with (
            tcx.tile_pool(name="w", bufs=1) as wp,
            tcx.tile_pool(name="psP", bufs=4, space="PSUM") as pp,
            tcx.tile_pool(name="bcast", bufs=4) as bcp,
        ):
            c.wp, c.pp, c.bcp = wp, pp, bcp
            _load_weights(c, nc, drams)
            c.xa = wp.tile([128, 3 + s], BF16, tag="xa", name="xa")
            c.xb = wp.tile([128, 3 + s], BF16, tag="xb", name="xb")
            nc.vector.memset(c.xa[:, 0:3], 0.0)
            nc.vector.memset(c.xb[:, 0:3], 0.0)

            with tcx.tile_pool(name="feat", bufs=1) as fp:
                c.w_feat = fp.tile([11, s], BF16, tag="featT", name="featT")
                nc.sync.dma_start(c.w_feat, drams[14][:])
                _embed(c, nc)

            with (
                tcx.tile_pool(name="hcp", bufs=4) as hcp,
                tcx.tile_pool(name="ubuf", bufs=2) as ubufp,
                tcx.tile_pool(name="zbuf", bufs=2) as zbufp,
                tcx.tile_pool(name="dbuf", bufs=2) as dbufp,
                tcx.tile_pool(name="dubuf", bufs=2) as dubufp,
                tcx.tile_pool(name="xdbl", bufs=2) as xdblp,
                tcx.tile_pool(name="ybuf", bufs=2) as ybufp,
                tcx.tile_pool(name="ygate", bufs=2) as ygatep,
                tcx.tile_pool(name="da", bufs=2) as dap,
                tcx.tile_pool(name="dbu", bufs=3) as dbup,
                tcx.tile_pool(name="hb", bufs=3) as hp,
                tcx.tile_pool(name="tmp", bufs=2) as tmpp,
            ):
                c.hcp = hcp
                c.ubufp, c.zbufp, c.dbufp, c.dubufp = ubufp, zbufp, dbufp, dubufp
                c.xdblp, c.ybufp, c.ygatep = xdblp, ybufp, ygatep
                c.dap, c.dbup, c.hp, c.tmpp = dap, dbup, hp, tmpp

                for rep in range(nloops):
                    for l in range(NL):
                        _layer(c, nc, l, bcd)
                _head(c, nc, out)

    nc.compile()
    return nc


_CACHE = {}


def _get_nc(s, tc_len, nloops=1):
    key = (s, tc_len, nloops)
    if key not in _CACHE:
        _CACHE[key] = build(s, tc_len, nloops)
    return _CACHE[key]


def _sel_matrix():
    """[40, 32*128]: slice i selects xd row 8+i (B_n, i<16) or 24+(i-16)."""
    sel = np.zeros((40, 32 * 128), dtype=np.float32)
    for i in range(32):
        row = RK + i if i < NST else RK + NST + (i - NST)
        sel[row, i * 128:(i + 1) * 128] = 1.0
    return sel


def prep_inputs(features, emb_w, emb_b, in_proj_w, conv_w, conv_b, x_proj_w,
                dt_w, dt_b, A_log, D, out_proj_w, head_w, head_b):
    """Host-side weight preprocessing shared by all cores."""
    import ml_dtypes
    f32 = np.float32
    bf16 = ml_dtypes.bfloat16

    nl = in_proj_w.shape[0]
    kuc = np.zeros((nl, 128, DC * DI), dtype=f32)
    for l in range(nl):
        wu = in_proj_w[l][:, :DI]                      # [128, 256]
        for k in range(DC):
            kuc[l][:, k * DI:(k + 1) * DI] = wu * conv_w[l][:, k][None, :]
    wz = in_proj_w[:, :, DI:]                          # [NL, 128, 256]
    xpw = np.zeros((nl, 128, 80), dtype=f32)
    ow = np.zeros((nl, 128, 256), dtype=f32)
    aneg = np.zeros((nl, 128, 2 * NST), dtype=f32)
    dtb2 = np.zeros((nl, 128, 2), dtype=f32)
    cb2 = np.zeros((nl, 128, 2), dtype=f32)
    dp2 = np.zeros((nl, 128, 2), dtype=f32)
    for l in range(nl):
        for ct in range(2):
            xpw[l][:, ct * 40:(ct + 1) * 40] = \
                x_proj_w[l][ct * 128:(ct + 1) * 128, :]
            ow[l][:, ct * 128:(ct + 1) * 128] = \
                out_proj_w[l][ct * 128:(ct + 1) * 128, :]
            aneg[l][:, ct * NST:(ct + 1) * NST] = \
                -np.exp(A_log[l][ct * 128:(ct + 1) * 128, :])
            dtb2[l][:, ct] = dt_b[l][ct * 128:(ct + 1) * 128]
            cb2[l][:, ct] = conv_b[l][ct * 128:(ct + 1) * 128]
            dp2[l][:, ct] = D[l][ct * 128:(ct + 1) * 128]

    return {
        "kuc": kuc.astype(bf16),
        "wz": np.ascontiguousarray(wz).astype(bf16),
        "xpw": xpw.astype(bf16),
        "dtw": np.ascontiguousarray(dt_w).astype(bf16),
        "ow": ow.astype(bf16),
        "emb": np.ascontiguousarray(emb_w).astype(bf16),
        "headw": np.ascontiguousarray(head_w).astype(bf16),
        "ones": _sel_matrix().astype(bf16),
        "dtb": dtb2,
        "cb": cb2,
        "aneg": aneg,
        "dpar": dp2,
        "embb": np.asarray(emb_b).reshape(128, 1).astype(f32),
        "headb": np.asarray(head_b).reshape(1, 1).astype(f32),
    }


def kernel(features, emb_w, emb_b, in_proj_w, conv_w, conv_b, x_proj_w,
           dt_w, dt_b, A_log, D, out_proj_w, head_w, head_b,
           _tc_len=1024, _trace=False):
    from concourse.bass_utils import run_bass_kernel_spmd
    import ml_dtypes

    args = [np.asarray(a) for a in (
        features, emb_w, emb_b, in_proj_w, conv_w, conv_b, x_proj_w,
        dt_w, dt_b, A_log, D, out_proj_w, head_w, head_b)]
    features = args[0]
    b, s, _ = features.shape
    assert b == NCORES
    nc = _get_nc(s, _tc_len)
    common = prep_inputs(*args)
    in_maps = []
    for i in range(NCORES):
        m = dict(common)
        m["featT"] = np.ascontiguousarray(
            features[i].T).astype(ml_dtypes.bfloat16)
        in_maps.append(m)
    res = run_bass_kernel_spmd(nc, in_maps, core_ids=list(range(NCORES)),
                               trace=_trace)
    out = np.stack([r["out"].reshape(s, 1) for r in res.results])
    kernel.last_result = res
    return out.astype(np.float32)

